# revision 31
# baseline (speedup 1.0000x reference)
"""Trainium2 Bass kernel for DeepGraphGO-style 2-layer GraphConv model.

  x1 = relu(features @ W1 + b1)
  x2 = GraphConv(x1; src1, dst1, Wc1, bc1)   # D_in^-1/2 A D_out^-1/2 x W + b
  x3 = GraphConv(x2; src2, dst2, Wc2, bc2)
  out = sigmoid(x3 @ W2 + b2)

Sharding: nodes are padded to 20480 and split contiguously across 8 cores
(2560 nodes per core, 20 blocks of 128).  Each core computes its node shard
through every layer; the per-layer "message" tensors g = (x @ Wc) * deg_out^-1/2
are quantized to fp8-e4m3 and AllGathered so every core can gather arbitrary
source rows (1 KB/row).

The per-edge gather is bound by SWDGE descriptor generation on the GpSimd Q7
(~10 ns/row, engine-serial), so the kernel is organized to keep that engine
busy continuously: each layer's AllGather is split into two node-half
collectives (first half fires as soon as the first 10 blocks' messages are
ready, overlapping the producing phase), and each destination block's edges
are host-sorted by (dst, src-half) so gathers for the first half start before
the second collective lands.

The segment-sum is computed per 128-node destination block as one-hot
selection matmuls on the tensor engine in fp8 DoubleRow mode (256 edges per
pass); one-hot matrices are built on-device (iota + is_equal on the vector
engine) from compact per-edge destination-column ids.  The final x3 @ W2 GEMM
is interleaved per block into the conv2 loop so tensor-engine work overlaps
gather DMA; output is written bf16 and upcast on host.
"""

import math
import os
from dataclasses import dataclass

import numpy as np
import ml_dtypes

import concourse.bass as bass
import concourse.bacc as bacc
import concourse.tile as tile
from concourse import mybir
from concourse.masks import make_identity
from concourse.bass_utils import run_bass_kernel_spmd

BF16 = ml_dtypes.bfloat16
FP8 = ml_dtypes.float8_e4m3
P = 128


@dataclass(frozen=True)
class Cfg:
    n_nodes: int = 20000          # real nodes
    n_cores: int = 8
    nb: int = 20                  # 128-node blocks per core
    fin: int = 2048               # input feature dim
    h: int = 1024                 # hidden dim
    go: int = 5000                # output dim

    @property
    def npc(self):                # nodes per core (padded)
        return self.nb * P

    npieces: int = 4              # AllGather pieces per layer

    @property
    def bpp(self):                # blocks per AG piece
        return self.nb // self.npieces

    @property
    def npp(self):                # nodes per core per AG piece
        return self.bpp * P

    @property
    def n_pad(self):
        return self.n_cores * self.npc

    @property
    def ki(self):                 # fin 128-chunks
        return self.fin // P

    @property
    def kh(self):                 # h 128-chunks
        return self.h // P


FULL = Cfg()


# ---------------------------------------------------------------- host prep

def _tile_kmaj(w, k_chunks, ncols):
    """[k_chunks*128, ncols] -> [128, k_chunks*ncols] with dev[p, k*ncols+j] = w[k*128+p, j]."""
    return np.ascontiguousarray(
        w.reshape(k_chunks, P, ncols).transpose(1, 0, 2).reshape(P, k_chunks * ncols)
    )


def _edge_prep(cfg, src, dst, cpb=None):
    """Per-core edge structures for one conv layer, with per-destination-block
    edges grouped by source node-half (half A: src%npc < nh).

    Returns (cpb, meta, per_core list of (idx_dev int16 [128, nb*cpb*8],
    dcol_dev f32 [128, nb*cpb])).  meta = (maxA, maxB, ncA, ncB): per-block
    max-over-cores real edge counts per half and chunk counts (shared across
    cores).  Gather row ids index the half buffer: core*nh + (src%npc) - half*nh.
    """
    npc, nb, npp, npieces = cfg.npc, cfg.nb, cfg.npp, cfg.npieces
    per_core = []
    for c in range(cfg.n_cores):
        sel = (dst >= c * npc) & (dst < (c + 1) * npc)
        s_e = src[sel].astype(np.int64)
        d_e = (dst[sel] - c * npc).astype(np.int64)
        piece = (s_e % npc) // npp
        order = np.lexsort((piece, d_e))
        s_e, d_e, piece = s_e[order], d_e[order], piece[order]
        blk = d_e // P
        cnts = [np.bincount(blk[piece == p], minlength=nb) for p in range(npieces)]
        per_core.append((s_e, d_e, piece, cnts))

    # per-(piece, block) max-over-cores real edge counts and chunk counts
    maxP = [[max(int(pc[3][p][b]) for pc in per_core) for b in range(nb)]
            for p in range(npieces)]
    ncP = [[math.ceil(m / P) for m in maxP[p]] for p in range(npieces)]
    ncSum = [sum(ncP[p][b] for p in range(npieces)) for b in range(nb)]
    # per-block chunk count padded even for DoubleRow pairing; the pad chunk
    # (and trailing pad rows of each piece) are never gathered - they are
    # DVE-memset to zero so stale NaN bytes can't poison the PE accumulation
    ncE = [n + n % 2 for n in ncSum]
    need_cpb = max(ncE)
    if cpb is None:
        cpb = need_cpb
    assert cpb >= need_cpb
    npad = cpb * P

    out = []
    for s_e, d_e, piece, cnts in per_core:
        idx_flat = np.full((nb, npad), -1, np.int64)     # -1: skipped by ucode
        dcol = np.full((nb, npad), -1.0, np.float32)     # -1 pad -> all-zero ws row
        for b in range(nb):
            mb_ = (d_e // P) == b
            base = 0
            for p in range(npieces):
                m = mb_ & (piece == p)
                cnt = int(m.sum())
                s_h = s_e[m]
                rows = (s_h // npc) * npp + (s_h % npc) - p * npp
                idx_flat[b, base:base + cnt] = rows
                idx_flat[b, base + cnt:base + maxP[p][b]] = 0   # shared-count pad
                dcol[b, base:base + cnt] = (d_e[m] - b * P).astype(np.float32)
                base += ncP[p][b] * P
        # dcol device layout: [128(edge lane), nb*cpb]; dev[p, b*cpb+j] = dcol[b, j*128+p]
        dcol_dev = np.ascontiguousarray(
            dcol.reshape(nb, cpb, P).transpose(2, 0, 1).reshape(P, nb * cpb)
        )
        # idx layout: wrapped into 16 partitions, replicated x8
        x = idx_flat.reshape(nb, cpb * 8, 16).transpose(2, 0, 1).reshape(16, nb * cpb * 8)
        idx_dev = np.ascontiguousarray(np.tile(x, (8, 1))).astype(np.int16)
        out.append((idx_dev, dcol_dev))
    return cpb, (maxP, ncP, ncE), out


def prep_inputs(cfg, inputs):
    """Build the SPMD per-core input maps. Returns (cpb, metas, in_maps)."""
    f32 = np.float32
    feats = np.asarray(inputs["features"], f32)
    W1 = np.asarray(inputs["W1"], f32)
    Wc1 = np.asarray(inputs["Wc1"], f32)
    Wc2 = np.asarray(inputs["Wc2"], f32)
    W2 = np.asarray(inputs["W2"], f32)
    for bname in ("b1", "bc1", "bc2", "b2"):
        assert not np.any(np.asarray(inputs[bname])), f"nonzero bias {bname} unsupported"
    src1 = np.asarray(inputs["src1"]).astype(np.int64)
    dst1 = np.asarray(inputs["dst1"]).astype(np.int64)
    src2 = np.asarray(inputs["src2"]).astype(np.int64)
    dst2 = np.asarray(inputs["dst2"]).astype(np.int64)

    npc, nb, n_pad = cfg.npc, cfg.nb, cfg.n_pad

    deg_out1 = np.maximum(np.bincount(src1, minlength=n_pad), 1.0).astype(f32) ** -0.5
    deg_in1 = np.maximum(np.bincount(dst1, minlength=n_pad), 1.0).astype(f32) ** -0.5
    deg_out2 = np.maximum(np.bincount(src2, minlength=n_pad), 1.0).astype(f32) ** -0.5
    deg_in2 = np.maximum(np.bincount(dst2, minlength=n_pad), 1.0).astype(f32) ** -0.5

    featp = np.zeros((n_pad, cfg.fin), f32)
    featp[: cfg.n_nodes] = feats

    w1_dev = _tile_kmaj(W1, cfg.ki, cfg.h).astype(BF16)
    wc1_dev = _tile_kmaj(Wc1, cfg.kh, cfg.h).astype(BF16)
    wc2_dev = _tile_kmaj(Wc2, cfg.kh, cfg.h).astype(BF16)
    w2_dev = _tile_kmaj(W2, cfg.kh, cfg.go).astype(FP8)

    cpb1, m1, e1 = _edge_prep(cfg, src1, dst1)
    cpb2, m2, e2 = _edge_prep(cfg, src2, dst2)
    cpb = max(cpb1, cpb2)
    if cpb1 < cpb:
        _, m1, e1 = _edge_prep(cfg, src1, dst1, cpb)
    if cpb2 < cpb:
        _, m2, e2 = _edge_prep(cfg, src2, dst2, cpb)

    in_maps = []
    for c in range(cfg.n_cores):
        lo, hi = c * npc, (c + 1) * npc
        featT = featp[lo:hi].T  # [fin, npc]
        featT_dev = _tile_kmaj(np.ascontiguousarray(featT), cfg.ki, npc).astype(BF16)
        s1 = deg_out1[lo:hi].reshape(nb, P).T                      # g1 row scale
        s2 = (deg_in1[lo:hi] * deg_out2[lo:hi]).reshape(nb, P).T   # g2 row scale
        s3 = deg_in2[lo:hi].reshape(nb, P).T                       # final scale
        s_all = np.ascontiguousarray(np.concatenate([s1, s2, s3], axis=1)).astype(f32)
        in_maps.append(
            {
                "featT": featT_dev,
                "w1": w1_dev,
                "wc1": wc1_dev,
                "wc2": wc2_dev,
                "w2": w2_dev,
                "s_all": s_all,
                "idx1": e1[c][0],
                "dcol1": e1[c][1],
                "idx2": e2[c][0],
                "dcol2": e2[c][1],
            }
        )
    return cpb, (m1, m2), in_maps


# ---------------------------------------------------------------- device build

def build_bass(cfg, cpb, metas, phases=4):
    f32, bf16, i16 = mybir.dt.float32, mybir.dt.bfloat16, mybir.dt.int16
    f8 = mybir.dt.float8e4
    nb, npc, ki, kh, h, go = cfg.nb, cfg.npc, cfg.ki, cfg.kh, cfg.h, cfg.go
    npp, bpp, npieces = cfg.npp, cfg.bpp, cfg.npieces
    ngrp = npc // 512

    nc = bacc.Bacc("TRN2", target_bir_lowering=False, debug=False, num_devices=cfg.n_cores)

    featT = nc.dram_tensor("featT", [P, ki * npc], bf16, kind="ExternalInput")
    w1 = nc.dram_tensor("w1", [P, ki * h], bf16, kind="ExternalInput")
    wc1 = nc.dram_tensor("wc1", [P, kh * h], bf16, kind="ExternalInput")
    wc2 = nc.dram_tensor("wc2", [P, kh * h], bf16, kind="ExternalInput")
    w2 = nc.dram_tensor("w2", [P, kh * go], f8, kind="ExternalInput")
    s_all = nc.dram_tensor("s_all", [P, 3 * nb], f32, kind="ExternalInput")
    idx1 = nc.dram_tensor("idx1", [P, nb * cpb * 8], i16, kind="ExternalInput")
    dcol1 = nc.dram_tensor("dcol1", [P, nb * cpb], f32, kind="ExternalInput")
    idx2 = nc.dram_tensor("idx2", [P, nb * cpb * 8], i16, kind="ExternalInput")
    dcol2 = nc.dram_tensor("dcol2", [P, nb * cpb], f32, kind="ExternalInput")
    out_d = nc.dram_tensor("out", [npc, go], bf16, kind="ExternalOutput")

    ag_in = {}
    ag_out = {}
    for layer in (1, 2):
        for p in range(npieces):
            ag_in[layer, p] = nc.dram_tensor(f"ag{layer}p{p}_in", [npp, h], f8, kind="Internal")
            ag_out[layer, p] = nc.dram_tensor(
                f"ag{layer}p{p}_out", [cfg.n_cores * npp, h], f8,
                kind="Internal", addr_space="Shared",
            )

    mult = mybir.AluOpType.mult
    is_eq = mybir.AluOpType.is_equal
    Relu = mybir.ActivationFunctionType.Relu
    Sigmoid = mybir.ActivationFunctionType.Sigmoid
    DR = mybir.MatmulPerfMode.DoubleRow
    rg = [list(range(cfg.n_cores))]

    # final-phase output column groups
    fgroups = []
    gstart = 0
    while gstart < go:
        gn = min(2048, go - gstart)
        fgroups.append((gstart, gn))
        gstart += gn

    def build_ws(ws, iota_w, dcol_sb, b):
        """One-hot scatter matrices for dst block b: ws[p, j, m] = (dcol[p, b*cpb+j] == m)."""
        nc.vector.tensor_tensor(
            out=ws[:], in0=iota_w[:],
            in1=dcol_sb[:, b * cpb:(b + 1) * cpb].broadcast_to([P, cpb, P]),
            op=is_eq,
        )

    gjn = int(os.environ.get("GNN_JN", "8"))

    def gather_piece(gt, ag_out_t, idx_sb, b, base, nch, mx):
        """Gather one source-piece of block b: chunks [base, base+nch) of gt,
        mx real rows (trailing -1 idx rows are skipped; their chunk was
        zeroed beforehand)."""
        for j0 in range(0, nch, gjn):
            jn = min(gjn, nch - j0)
            cnt = min(mx - j0 * P, jn * P)
            nc.gpsimd.dma_gather(
                gt[:, base + j0:base + j0 + jn, :].bitcast(bf16),
                ag_out_t[:].bitcast(bf16),
                idx_sb[:, (b * cpb + base + j0) * 8:(b * cpb + base + j0 + jn) * 8],
                jn * P, cnt, h // 2,
            )

    def gather_block(gt, layer, idx_sb, b, maxP, ncP, ncE):
        """All pieces of block b, zeroing partially-gathered chunks first."""
        base = 0
        for p in range(npieces):
            nch, mx = ncP[p][b], maxP[p][b]
            if nch:
                if mx < nch * P:
                    nc.vector.memset(gt[:, base + nch - 1, :], 0.0)
                gather_piece(gt, ag_out[layer, p], idx_sb, b, base, nch, mx)
            base += nch
        if base < ncE[b]:
            nc.vector.memset(gt[:, base:ncE[b], :], 0.0)

    def conv_block(gt, ws, ident, xb, nc_b, cps_p, tps_p, agg_p):
        """One dst block: DoubleRow scatter matmuls + transpose to feature-major xb."""
        npair = nc_b // 2
        ps = cps_p.tile([P, h], f32, tag="cps")
        for jp in range(npair):
            for hh in range(h // 512):
                nc.tensor.matmul(
                    ps[:, hh * 512:(hh + 1) * 512],
                    lhsT=ws[:, 2 * jp:2 * jp + 2, :],
                    rhs=gt[:, 2 * jp:2 * jp + 2, hh * 512:(hh + 1) * 512],
                    start=(jp == 0),
                    stop=(jp == npair - 1),
                    perf_mode=DR,
                )
        agg = agg_p.tile([P, h], bf16, tag="agg")
        nc.vector.tensor_copy(out=agg[:], in_=ps[:])
        for m in range(kh):
            tp = tps_p.tile([P, P], bf16, tag="tps")
            nc.tensor.transpose(out=tp[:], in_=agg[:, m * P:(m + 1) * P], identity=ident[:])
            nc.vector.tensor_copy(out=xb[:, m, :], in_=tp[:])

    with tile.TileContext(nc) as tc:
        with tc.tile_pool(name="consts", bufs=1) as consts:
            s_sb = consts.tile([P, 3 * nb], f32)
            nc.sync.dma_start(out=s_sb[:], in_=s_all[:])
            idx1_sb = consts.tile([P, nb * cpb * 8], i16)
            nc.sync.dma_start(out=idx1_sb[:], in_=idx1[:])
            idx2_sb = consts.tile([P, nb * cpb * 8], i16)
            nc.sync.dma_start(out=idx2_sb[:], in_=idx2[:])
            dcol1_sb = consts.tile([P, nb * cpb], f32)
            nc.sync.dma_start(out=dcol1_sb[:], in_=dcol1[:])
            dcol2_sb = consts.tile([P, nb * cpb], f32)
            nc.sync.dma_start(out=dcol2_sb[:], in_=dcol2[:])
            ident = consts.tile([P, P], bf16)
            make_identity(nc, ident[:])
            # iota_w[p, j, m] = m  (f32; values 0..127 are exact)
            iota_w = consts.tile([P, cpb, P], f32)
            nc.gpsimd.iota(
                iota_w[:], pattern=[[0, cpb], [1, P]], base=0,
                channel_multiplier=0, allow_small_or_imprecise_dtypes=True,
            )

            # ------------- phase 1: x1 = relu(W1^T featT) by 512-col groups;
            # g1[b] = (x1[b] @ Wc1) * s1[b] interleaved per 4-block group
            with tc.tile_pool(name="ph1", bufs=1) as ph1, \
                 tc.tile_pool(name="ft", bufs=2) as ft_p, \
                 tc.tile_pool(name="h1g", bufs=2) as h1g_p, \
                 tc.tile_pool(name="ps1", bufs=4, space="PSUM") as ps1_p, \
                 tc.tile_pool(name="gps1", bufs=1, space="PSUM") as gps1_p, \
                 tc.tile_pool(name="gout", bufs=2) as gout_p:
                w1_sb = ph1.tile([P, ki, h], bf16)
                nc.sync.dma_start(out=w1_sb[:], in_=w1[:].rearrange("p (k n) -> p k n", k=ki))
                wc1_sb = ph1.tile([P, kh, h], bf16)
                nc.sync.dma_start(out=wc1_sb[:], in_=wc1[:].rearrange("p (k n) -> p k n", k=kh))
                featT_r = featT[:].rearrange("p (k n) -> p k n", k=ki)
                for g in range(ngrp):
                    ft = ft_p.tile([P, ki, 512], bf16, tag="ft")
                    nc.sync.dma_start(out=ft[:], in_=featT_r[:, :, g * 512:(g + 1) * 512])
                    h1g = h1g_p.tile([P, kh, 512], bf16, tag="h1g")
                    for m in range(kh):
                        ps = ps1_p.tile([P, 512], f32, tag="ps1")
                        for k in range(ki):
                            nc.tensor.matmul(
                                ps[:],
                                lhsT=w1_sb[:, k, m * P:(m + 1) * P],
                                rhs=ft[:, k, :],
                                start=(k == 0),
                                stop=(k == ki - 1),
                            )
                        nc.scalar.activation(out=h1g[:, m, :], in_=ps[:], func=Relu)
                    for bq in range(4):
                        b = g * 4 + bq
                        ps2 = gps1_p.tile([P, h], f32, tag="gps")
                        for k in range(kh):
                            for hh in range(h // 512):
                                nc.tensor.matmul(
                                    ps2[:, hh * 512:(hh + 1) * 512],
                                    lhsT=h1g[:, k, bq * P:(bq + 1) * P],
                                    rhs=wc1_sb[:, k, hh * 512:(hh + 1) * 512],
                                    start=(k == 0),
                                    stop=(k == kh - 1),
                                )
                        gsb = gout_p.tile([P, h], f8, tag="gsb")
                        nc.vector.tensor_scalar(
                            out=gsb[:], in0=ps2[:], scalar1=s_sb[:, b:b + 1],
                            scalar2=None, op0=mult,
                        )
                        dst_t = ag_in[1, b // bpp]
                        roff = (b % bpp) * P
                        nc.sync.dma_start(out=dst_t[roff:roff + P, :], in_=gsb[:])

            # piece collectives fire as their 5-block spans complete
            for p in range(npieces):
                nc.gpsimd.collective_compute(
                    "AllGather", mybir.AluOpType.bypass,
                    ins=[ag_in[1, p][:]], outs=[ag_out[1, p][:]], replica_groups=rg,
                )

            # ------------- phases 2-4 share the resident W2 tile
            if phases >= 2:
                (maxP1, ncP1, ncE1), (maxP2, ncP2, ncE2) = metas
                with tc.tile_pool(name="ph234", bufs=1) as ph234:
                    w2_sb = ph234.tile([P, kh, go], f8)
                    w2_r = w2[:].rearrange("p (k n) -> p k n", k=kh)
                    for gstart, gn in fgroups:
                        nc.sync.dma_start(
                            out=w2_sb[:, :, gstart:gstart + gn],
                            in_=w2_r[:, :, gstart:gstart + gn],
                        )

                    # ----- phase 2: conv1 per block -> x2[b]; g2[b] = (x2[b] @ Wc2) * s2[b]
                    with tc.tile_pool(name="ph2", bufs=1) as ph2, \
                         tc.tile_pool(name="gat", bufs=2) as gat_p, \
                         tc.tile_pool(name="wsl", bufs=2) as wsl_p, \
                         tc.tile_pool(name="agg", bufs=2) as agg_p, \
                         tc.tile_pool(name="x2b", bufs=3) as x2b_p, \
                         tc.tile_pool(name="gout2", bufs=2) as gout2_p, \
                         tc.tile_pool(name="cps", bufs=2, space="PSUM") as cps_p, \
                         tc.tile_pool(name="tps", bufs=2, space="PSUM") as tps_p, \
                         tc.tile_pool(name="gps2", bufs=1, space="PSUM") as gps2_p:
                        wc2_sb = ph2.tile([P, kh, h], bf16)
                        nc.sync.dma_start(out=wc2_sb[:], in_=wc2[:].rearrange("p (k n) -> p k n", k=kh))
                        for b in range(nb):
                            gt = gat_p.tile([P, cpb, h], f8, tag="gt")
                            gather_block(gt, 1, idx1_sb, b, maxP1, ncP1, ncE1)
                            ws = wsl_p.tile([P, cpb, P], f8, tag="ws")
                            build_ws(ws, iota_w, dcol1_sb, b)
                            x2b = x2b_p.tile([P, kh, P], bf16, tag="x2b")
                            conv_block(gt, ws, ident, x2b, ncE1[b], cps_p, tps_p, agg_p)
                            ps2 = gps2_p.tile([P, h], f32, tag="g2ps")
                            for k in range(kh):
                                for hh in range(h // 512):
                                    nc.tensor.matmul(
                                        ps2[:, hh * 512:(hh + 1) * 512],
                                        lhsT=x2b[:, k, :],
                                        rhs=wc2_sb[:, k, hh * 512:(hh + 1) * 512],
                                        start=(k == 0),
                                        stop=(k == kh - 1),
                                    )
                            gsb = gout2_p.tile([P, h], f8, tag="gsb2")
                            nc.vector.tensor_scalar(
                                out=gsb[:], in0=ps2[:], scalar1=s_sb[:, nb + b:nb + b + 1],
                                scalar2=None, op0=mult,
                            )
                            dst_t = ag_in[2, b // bpp]
                            roff = (b % bpp) * P
                            nc.sync.dma_start(out=dst_t[roff:roff + P, :], in_=gsb[:])

                    for p in range(npieces):
                        nc.gpsimd.collective_compute(
                            "AllGather", mybir.AluOpType.bypass,
                            ins=[ag_in[2, p][:]], outs=[ag_out[2, p][:]], replica_groups=rg,
                        )

                    # ----- phase 3+4: conv2 per block -> x3[b]; out[b] = sigmoid(s3*(x3[b] @ W2))
                    if phases >= 3:
                        with tc.tile_pool(name="gat3", bufs=2) as gat3_p, \
                             tc.tile_pool(name="wsl3", bufs=2) as wsl3_p, \
                             tc.tile_pool(name="agg3", bufs=2) as agg3_p, \
                             tc.tile_pool(name="x3b", bufs=3) as x3b_p, \
                             tc.tile_pool(name="fout", bufs=3) as fout_p, \
                             tc.tile_pool(name="cps3", bufs=2, space="PSUM") as cps3_p, \
                             tc.tile_pool(name="tps3", bufs=2, space="PSUM") as tps3_p, \
                             tc.tile_pool(name="fps", bufs=2, space="PSUM") as fps_p:
                            for b in range(nb):
                                gt = gat3_p.tile([P, cpb, h], f8, tag="gt3")
                                gather_block(gt, 2, idx2_sb, b, maxP2, ncP2, ncE2)
                                ws = wsl3_p.tile([P, cpb, P], f8, tag="ws3")
                                build_ws(ws, iota_w, dcol2_sb, b)
                                x3b = x3b_p.tile([P, kh, P], f8, tag="x3b")
                                conv_block(gt, ws, ident, x3b, ncE2[b], cps3_p, tps3_p, agg3_p)
                                if phases >= 4:
                                    for gstart, gn in fgroups:
                                        o = fout_p.tile([P, 2048], bf16, tag="fo")
                                        for cs in range(0, gn, 512):
                                            cn = min(512, gn - cs)
                                            ps4 = fps_p.tile([P, 512], f32, tag="fps")
                                            for k2 in range(0, kh, 2):
                                                nc.tensor.matmul(
                                                    ps4[:, :cn],
                                                    lhsT=x3b[:, k2:k2 + 2, :],
                                                    rhs=w2_sb[:, k2:k2 + 2, gstart + cs:gstart + cs + cn],
                                                    start=(k2 == 0),
                                                    stop=(k2 == kh - 2),
                                                    perf_mode=DR,
                                                )
                                            nc.scalar.activation(
                                                out=o[:, cs:cs + cn], in_=ps4[:, :cn], func=Sigmoid,
                                                scale=s_sb[:, 2 * nb + b:2 * nb + b + 1],
                                            )
                                        nc.sync.dma_start(
                                            out=out_d[b * P:(b + 1) * P, gstart:gstart + gn],
                                            in_=o[:, :gn],
                                        )

    nc.compile()
    return nc


# ---------------------------------------------------------------- entry point

def _ensure_ntff_hook():
    """Register the axon NTFF profile hook if the image's antenv lacks it."""
    import contextlib
    import ctypes
    import sys
    import types

    try:
        from antenv.axon_hooks import get_axon_ntff_profile_hook  # noqa: F401
        return
    except ImportError:
        pass
    try:
        import antenv
    except ImportError:
        return
    mod = types.ModuleType("antenv.axon_hooks")
    holder = [None]
    mod.set_axon_ntff_profile_hook = lambda h: holder.__setitem__(0, h)
    mod.get_axon_ntff_profile_hook = lambda: holder[0]
    sys.modules["antenv.axon_hooks"] = mod
    antenv.axon_hooks = mod
    try:
        lib = ctypes.CDLL("/opt/axon/libaxon_pjrt.so")
    except OSError:
        return
    if not hasattr(lib, "axon_start_nrt_profile"):
        return
    lib.axon_start_nrt_profile.argtypes = [
        ctypes.POINTER(ctypes.c_int64),
        ctypes.c_size_t,
    ]
    lib.axon_start_nrt_profile.restype = ctypes.c_int64
    lib.axon_stop_nrt_profile.argtypes = [ctypes.c_char_p]
    lib.axon_stop_nrt_profile.restype = ctypes.c_int64

    @contextlib.contextmanager
    def _hook(output_dir, device_ids):
        import jax

        jax.devices()
        if device_ids:
            ids = (ctypes.c_int64 * len(device_ids))(*device_ids)
            rc = lib.axon_start_nrt_profile(ids, len(device_ids))
        else:
            rc = lib.axon_start_nrt_profile(None, 0)
        if rc != 0:
            raise RuntimeError(f"axon_start_nrt_profile rc={rc}")
        try:
            yield
        finally:
            n = lib.axon_stop_nrt_profile(str(output_dir).encode())
            print(f"profile: {n} file(s) written to {output_dir}", file=sys.stderr)

    holder[0] = _hook


def _run_hw(cfg, inputs, trace=False):
    if trace:
        _ensure_ntff_hook()
    cpb, metas, in_maps = prep_inputs(cfg, inputs)
    phases = int(os.environ.get("GNN_PHASES", "4"))
    nc = build_bass(cfg, cpb, metas, phases=phases)
    res = run_bass_kernel_spmd(nc, in_maps, core_ids=list(range(cfg.n_cores)), trace=trace)
    full = np.concatenate(
        [np.asarray(res.results[c]["out"]).astype(np.float32) for c in range(cfg.n_cores)],
        axis=0,
    )
    return full[: cfg.n_nodes], res


def kernel(**inputs) -> np.ndarray:
    trace = bool(int(os.environ.get("GNN_TRACE", "0")))
    out, res = _run_hw(FULL, inputs, trace=trace)
    if trace and res.exec_time_ns is not None:
        print(f"HW exec time: {res.exec_time_ns} ns")
    return out


# revision 32
# speedup vs baseline: 1.1420x; 1.1420x over previous
"""Trainium2 Bass kernel for DeepGraphGO-style 2-layer GraphConv model.

  x1 = relu(features @ W1 + b1)
  x2 = GraphConv(x1; src1, dst1, Wc1, bc1)   # D_in^-1/2 A D_out^-1/2 x W + b
  x3 = GraphConv(x2; src2, dst2, Wc2, bc2)
  out = sigmoid(x3 @ W2 + b2)

Sharding: nodes are padded to 20480 and split contiguously across 8 cores
(2560 nodes per core, 20 blocks of 128).  Each core computes its node shard
through every layer; the per-layer "message" tensors g = (x @ Wc) * deg_out^-1/2
are quantized to fp8-e4m3 and AllGathered so every core can gather arbitrary
source rows (1 KB/row).

The per-edge gather is bound by SWDGE descriptor generation on the GpSimd Q7
(~10 ns/row, engine-serial), so the kernel is organized to keep that engine
busy continuously: each layer's AllGather is split into two node-half
collectives (first half fires as soon as the first 10 blocks' messages are
ready, overlapping the producing phase), and each destination block's edges
are host-sorted by (dst, src-half) so gathers for the first half start before
the second collective lands.

The segment-sum is computed per 128-node destination block as one-hot
selection matmuls on the tensor engine in fp8 DoubleRow mode (256 edges per
pass); one-hot matrices are built on-device (iota + is_equal on the vector
engine) from compact per-edge destination-column ids.  The final x3 @ W2 GEMM
is interleaved per block into the conv2 loop so tensor-engine work overlaps
gather DMA; output is written bf16 and upcast on host.
"""

import math
import os
from dataclasses import dataclass

import numpy as np
import ml_dtypes

import concourse.bass as bass
import concourse.bacc as bacc
import concourse.tile as tile
from concourse import mybir
from concourse.masks import make_identity
from concourse.bass_utils import run_bass_kernel_spmd

BF16 = ml_dtypes.bfloat16
FP8 = ml_dtypes.float8_e4m3
P = 128


@dataclass(frozen=True)
class Cfg:
    n_nodes: int = 20000          # real nodes
    n_cores: int = 8
    nb: int = 20                  # 128-node blocks per core
    fin: int = 2048               # input feature dim
    h: int = 1024                 # hidden dim
    go: int = 5000                # output dim

    @property
    def npc(self):                # nodes per core (padded)
        return self.nb * P

    npieces: int = 4              # AllGather pieces per layer

    @property
    def bpp(self):                # blocks per AG piece
        return self.nb // self.npieces

    @property
    def npp(self):                # nodes per core per AG piece
        return self.bpp * P

    @property
    def n_pad(self):
        return self.n_cores * self.npc

    @property
    def ki(self):                 # fin 128-chunks
        return self.fin // P

    @property
    def kh(self):                 # h 128-chunks
        return self.h // P


FULL = Cfg()


# ---------------------------------------------------------------- host prep

def _tile_kmaj(w, k_chunks, ncols):
    """[k_chunks*128, ncols] -> [128, k_chunks*ncols] with dev[p, k*ncols+j] = w[k*128+p, j]."""
    return np.ascontiguousarray(
        w.reshape(k_chunks, P, ncols).transpose(1, 0, 2).reshape(P, k_chunks * ncols)
    )


def _edge_prep(cfg, src, dst, cpb=None):
    """Per-core edge structures for one conv layer, with per-destination-block
    edges grouped by source node-half (half A: src%npc < nh).

    Returns (cpb, meta, per_core list of (idx_dev int16 [128, nb*cpb*8],
    dcol_dev f32 [128, nb*cpb])).  meta = (maxA, maxB, ncA, ncB): per-block
    max-over-cores real edge counts per half and chunk counts (shared across
    cores).  Gather row ids index the half buffer: core*nh + (src%npc) - half*nh.
    """
    npc, nb, npp, npieces = cfg.npc, cfg.nb, cfg.npp, cfg.npieces
    per_core = []
    for c in range(cfg.n_cores):
        sel = (dst >= c * npc) & (dst < (c + 1) * npc)
        s_e = src[sel].astype(np.int64)
        d_e = (dst[sel] - c * npc).astype(np.int64)
        piece = (s_e % npc) // npp
        order = np.lexsort((piece, d_e))
        s_e, d_e, piece = s_e[order], d_e[order], piece[order]
        blk = d_e // P
        cnts = [np.bincount(blk[piece == p], minlength=nb) for p in range(npieces)]
        per_core.append((s_e, d_e, piece, cnts))

    # per-(piece, block) max-over-cores real edge counts and chunk counts
    maxP = [[max(int(pc[3][p][b]) for pc in per_core) for b in range(nb)]
            for p in range(npieces)]
    ncP = [[math.ceil(m / P) for m in maxP[p]] for p in range(npieces)]
    ncSum = [sum(ncP[p][b] for p in range(npieces)) for b in range(nb)]
    # per-block chunk count padded even for DoubleRow pairing; the pad chunk
    # is folded into the last piece and gathered as row-0 dummies so every
    # chunk the matmul reads holds finite data
    ncE = [n + n % 2 for n in ncSum]
    for b in range(nb):
        ncP[npieces - 1][b] += ncE[b] - ncSum[b]
    need_cpb = max(ncE)
    if cpb is None:
        cpb = need_cpb
    assert cpb >= need_cpb
    npad = cpb * P

    out = []
    for s_e, d_e, piece, cnts in per_core:
        idx_flat = np.full((nb, npad), -1, np.int64)     # -1: skipped by ucode
        dcol = np.full((nb, npad), -1.0, np.float32)     # -1 pad -> all-zero ws row
        for b in range(nb):
            mb_ = (d_e // P) == b
            base = 0
            for p in range(npieces):
                m = mb_ & (piece == p)
                cnt = int(m.sum())
                s_h = s_e[m]
                rows = (s_h // npc) * npp + (s_h % npc) - p * npp
                idx_flat[b, base:base + cnt] = rows
                idx_flat[b, base + cnt:base + ncP[p][b] * P] = 0   # pad: row 0 (finite)
                dcol[b, base:base + cnt] = (d_e[m] - b * P).astype(np.float32)
                base += ncP[p][b] * P
        # dcol device layout: [128(edge lane), nb*cpb]; dev[p, b*cpb+j] = dcol[b, j*128+p]
        dcol_dev = np.ascontiguousarray(
            dcol.reshape(nb, cpb, P).transpose(2, 0, 1).reshape(P, nb * cpb)
        )
        # idx layout: wrapped into 16 partitions, replicated x8
        x = idx_flat.reshape(nb, cpb * 8, 16).transpose(2, 0, 1).reshape(16, nb * cpb * 8)
        idx_dev = np.ascontiguousarray(np.tile(x, (8, 1))).astype(np.int16)
        out.append((idx_dev, dcol_dev))
    return cpb, (maxP, ncP, ncE), out


def prep_inputs(cfg, inputs):
    """Build the SPMD per-core input maps. Returns (cpb, metas, in_maps)."""
    f32 = np.float32
    feats = np.asarray(inputs["features"], f32)
    W1 = np.asarray(inputs["W1"], f32)
    Wc1 = np.asarray(inputs["Wc1"], f32)
    Wc2 = np.asarray(inputs["Wc2"], f32)
    W2 = np.asarray(inputs["W2"], f32)
    for bname in ("b1", "bc1", "bc2", "b2"):
        assert not np.any(np.asarray(inputs[bname])), f"nonzero bias {bname} unsupported"
    src1 = np.asarray(inputs["src1"]).astype(np.int64)
    dst1 = np.asarray(inputs["dst1"]).astype(np.int64)
    src2 = np.asarray(inputs["src2"]).astype(np.int64)
    dst2 = np.asarray(inputs["dst2"]).astype(np.int64)

    npc, nb, n_pad = cfg.npc, cfg.nb, cfg.n_pad

    deg_out1 = np.maximum(np.bincount(src1, minlength=n_pad), 1.0).astype(f32) ** -0.5
    deg_in1 = np.maximum(np.bincount(dst1, minlength=n_pad), 1.0).astype(f32) ** -0.5
    deg_out2 = np.maximum(np.bincount(src2, minlength=n_pad), 1.0).astype(f32) ** -0.5
    deg_in2 = np.maximum(np.bincount(dst2, minlength=n_pad), 1.0).astype(f32) ** -0.5

    featp = np.zeros((n_pad, cfg.fin), f32)
    featp[: cfg.n_nodes] = feats

    w1_dev = _tile_kmaj(W1, cfg.ki, cfg.h).astype(BF16)
    wc1_dev = _tile_kmaj(Wc1, cfg.kh, cfg.h).astype(BF16)
    wc2_dev = _tile_kmaj(Wc2, cfg.kh, cfg.h).astype(BF16)
    w2_dev = _tile_kmaj(W2, cfg.kh, cfg.go).astype(FP8)

    cpb1, m1, e1 = _edge_prep(cfg, src1, dst1)
    cpb2, m2, e2 = _edge_prep(cfg, src2, dst2)
    cpb = max(cpb1, cpb2)
    if cpb1 < cpb:
        _, m1, e1 = _edge_prep(cfg, src1, dst1, cpb)
    if cpb2 < cpb:
        _, m2, e2 = _edge_prep(cfg, src2, dst2, cpb)

    in_maps = []
    for c in range(cfg.n_cores):
        lo, hi = c * npc, (c + 1) * npc
        featT = featp[lo:hi].T  # [fin, npc]
        featT_dev = _tile_kmaj(np.ascontiguousarray(featT), cfg.ki, npc).astype(BF16)
        s1 = deg_out1[lo:hi].reshape(nb, P).T                      # g1 row scale
        s2 = (deg_in1[lo:hi] * deg_out2[lo:hi]).reshape(nb, P).T   # g2 row scale
        s3 = deg_in2[lo:hi].reshape(nb, P).T                       # final scale
        s_all = np.ascontiguousarray(np.concatenate([s1, s2, s3], axis=1)).astype(f32)
        in_maps.append(
            {
                "featT": featT_dev,
                "w1": w1_dev,
                "wc1": wc1_dev,
                "wc2": wc2_dev,
                "w2": w2_dev,
                "s_all": s_all,
                "idx1": e1[c][0],
                "dcol1": e1[c][1],
                "idx2": e2[c][0],
                "dcol2": e2[c][1],
            }
        )
    return cpb, (m1, m2), in_maps


# ---------------------------------------------------------------- device build

def build_bass(cfg, cpb, metas, phases=4):
    f32, bf16, i16 = mybir.dt.float32, mybir.dt.bfloat16, mybir.dt.int16
    f8 = mybir.dt.float8e4
    nb, npc, ki, kh, h, go = cfg.nb, cfg.npc, cfg.ki, cfg.kh, cfg.h, cfg.go
    npp, bpp, npieces = cfg.npp, cfg.bpp, cfg.npieces
    ngrp = npc // 512

    nc = bacc.Bacc("TRN2", target_bir_lowering=False, debug=False, num_devices=cfg.n_cores)

    featT = nc.dram_tensor("featT", [P, ki * npc], bf16, kind="ExternalInput")
    w1 = nc.dram_tensor("w1", [P, ki * h], bf16, kind="ExternalInput")
    wc1 = nc.dram_tensor("wc1", [P, kh * h], bf16, kind="ExternalInput")
    wc2 = nc.dram_tensor("wc2", [P, kh * h], bf16, kind="ExternalInput")
    w2 = nc.dram_tensor("w2", [P, kh * go], f8, kind="ExternalInput")
    s_all = nc.dram_tensor("s_all", [P, 3 * nb], f32, kind="ExternalInput")
    idx1 = nc.dram_tensor("idx1", [P, nb * cpb * 8], i16, kind="ExternalInput")
    dcol1 = nc.dram_tensor("dcol1", [P, nb * cpb], f32, kind="ExternalInput")
    idx2 = nc.dram_tensor("idx2", [P, nb * cpb * 8], i16, kind="ExternalInput")
    dcol2 = nc.dram_tensor("dcol2", [P, nb * cpb], f32, kind="ExternalInput")
    out_d = nc.dram_tensor("out", [npc, go], bf16, kind="ExternalOutput")

    ag_in = {}
    ag_out = {}
    for layer in (1, 2):
        for p in range(npieces):
            ag_in[layer, p] = nc.dram_tensor(f"ag{layer}p{p}_in", [npp, h], f8, kind="Internal")
            ag_out[layer, p] = nc.dram_tensor(
                f"ag{layer}p{p}_out", [cfg.n_cores * npp, h], f8,
                kind="Internal", addr_space="Shared",
            )

    mult = mybir.AluOpType.mult
    is_eq = mybir.AluOpType.is_equal
    Relu = mybir.ActivationFunctionType.Relu
    Sigmoid = mybir.ActivationFunctionType.Sigmoid
    DR = mybir.MatmulPerfMode.DoubleRow
    rg = [list(range(cfg.n_cores))]

    # final-phase output column groups
    fgroups = []
    gstart = 0
    while gstart < go:
        gn = min(2048, go - gstart)
        fgroups.append((gstart, gn))
        gstart += gn

    def build_ws(ws, iota_w, dcol_sb, b):
        """One-hot scatter matrices for dst block b: ws[p, j, m] = (dcol[p, b*cpb+j] == m)."""
        nc.vector.tensor_tensor(
            out=ws[:], in0=iota_w[:],
            in1=dcol_sb[:, b * cpb:(b + 1) * cpb].broadcast_to([P, cpb, P]),
            op=is_eq,
        )

    gjn = int(os.environ.get("GNN_JN", "8"))

    def gather_piece(gt, ag_out_t, idx_sb, b, base, nch):
        """Gather one source-piece of block b: chunks [base, base+nch) of gt."""
        for j0 in range(0, nch, gjn):
            jn = min(gjn, nch - j0)
            nc.gpsimd.dma_gather(
                gt[:, base + j0:base + j0 + jn, :].bitcast(bf16),
                ag_out_t[:].bitcast(bf16),
                idx_sb[:, (b * cpb + base + j0) * 8:(b * cpb + base + j0 + jn) * 8],
                jn * P, jn * P, h // 2,
            )

    def gather_block(gt, layer, idx_sb, b, maxP, ncP, ncE):
        """All pieces of block b (pad rows gather row 0 - always finite)."""
        base = 0
        for p in range(npieces):
            nch = ncP[p][b]
            if nch:
                gather_piece(gt, ag_out[layer, p], idx_sb, b, base, nch)
            base += nch

    def conv_block(gt, ws, ident, xb, nc_b, cps_p, tps_p, agg_p):
        """One dst block: DoubleRow scatter matmuls + transpose to feature-major xb."""
        npair = nc_b // 2
        ps = cps_p.tile([P, h], f32, tag="cps")
        for jp in range(npair):
            for hh in range(h // 512):
                nc.tensor.matmul(
                    ps[:, hh * 512:(hh + 1) * 512],
                    lhsT=ws[:, 2 * jp:2 * jp + 2, :],
                    rhs=gt[:, 2 * jp:2 * jp + 2, hh * 512:(hh + 1) * 512],
                    start=(jp == 0),
                    stop=(jp == npair - 1),
                    perf_mode=DR,
                )
        agg = agg_p.tile([P, h], bf16, tag="agg")
        nc.vector.tensor_copy(out=agg[:], in_=ps[:])
        for m in range(kh):
            tp = tps_p.tile([P, P], bf16, tag="tps")
            nc.tensor.transpose(out=tp[:], in_=agg[:, m * P:(m + 1) * P], identity=ident[:])
            nc.vector.tensor_copy(out=xb[:, m, :], in_=tp[:])

    with tile.TileContext(nc) as tc:
        with tc.tile_pool(name="consts", bufs=1) as consts:
            s_sb = consts.tile([P, 3 * nb], f32)
            nc.sync.dma_start(out=s_sb[:], in_=s_all[:])
            idx1_sb = consts.tile([P, nb * cpb * 8], i16)
            nc.sync.dma_start(out=idx1_sb[:], in_=idx1[:])
            idx2_sb = consts.tile([P, nb * cpb * 8], i16)
            nc.sync.dma_start(out=idx2_sb[:], in_=idx2[:])
            dcol1_sb = consts.tile([P, nb * cpb], f32)
            nc.sync.dma_start(out=dcol1_sb[:], in_=dcol1[:])
            dcol2_sb = consts.tile([P, nb * cpb], f32)
            nc.sync.dma_start(out=dcol2_sb[:], in_=dcol2[:])
            ident = consts.tile([P, P], bf16)
            make_identity(nc, ident[:])
            # iota_w[p, j, m] = m  (f32; values 0..127 are exact)
            iota_w = consts.tile([P, cpb, P], f32)
            nc.gpsimd.iota(
                iota_w[:], pattern=[[0, cpb], [1, P]], base=0,
                channel_multiplier=0, allow_small_or_imprecise_dtypes=True,
            )

            # ------------- phase 1: x1 = relu(W1^T featT) by 512-col groups;
            # g1[b] = (x1[b] @ Wc1) * s1[b] interleaved per 4-block group
            with tc.tile_pool(name="ph1", bufs=1) as ph1, \
                 tc.tile_pool(name="ft", bufs=2) as ft_p, \
                 tc.tile_pool(name="h1g", bufs=2) as h1g_p, \
                 tc.tile_pool(name="ps1", bufs=4, space="PSUM") as ps1_p, \
                 tc.tile_pool(name="gps1", bufs=1, space="PSUM") as gps1_p, \
                 tc.tile_pool(name="gout", bufs=2) as gout_p:
                w1_sb = ph1.tile([P, ki, h], bf16)
                nc.sync.dma_start(out=w1_sb[:], in_=w1[:].rearrange("p (k n) -> p k n", k=ki))
                wc1_sb = ph1.tile([P, kh, h], bf16)
                nc.sync.dma_start(out=wc1_sb[:], in_=wc1[:].rearrange("p (k n) -> p k n", k=kh))
                featT_r = featT[:].rearrange("p (k n) -> p k n", k=ki)
                for g in range(ngrp):
                    ft = ft_p.tile([P, ki, 512], bf16, tag="ft")
                    nc.sync.dma_start(out=ft[:], in_=featT_r[:, :, g * 512:(g + 1) * 512])
                    h1g = h1g_p.tile([P, kh, 512], bf16, tag="h1g")
                    for m in range(kh):
                        ps = ps1_p.tile([P, 512], f32, tag="ps1")
                        for k in range(ki):
                            nc.tensor.matmul(
                                ps[:],
                                lhsT=w1_sb[:, k, m * P:(m + 1) * P],
                                rhs=ft[:, k, :],
                                start=(k == 0),
                                stop=(k == ki - 1),
                            )
                        nc.scalar.activation(out=h1g[:, m, :], in_=ps[:], func=Relu)
                    for bq in range(4):
                        b = g * 4 + bq
                        ps2 = gps1_p.tile([P, h], f32, tag="gps")
                        for k in range(kh):
                            for hh in range(h // 512):
                                nc.tensor.matmul(
                                    ps2[:, hh * 512:(hh + 1) * 512],
                                    lhsT=h1g[:, k, bq * P:(bq + 1) * P],
                                    rhs=wc1_sb[:, k, hh * 512:(hh + 1) * 512],
                                    start=(k == 0),
                                    stop=(k == kh - 1),
                                )
                        gsb = gout_p.tile([P, h], f8, tag="gsb")
                        nc.vector.tensor_scalar(
                            out=gsb[:], in0=ps2[:], scalar1=s_sb[:, b:b + 1],
                            scalar2=None, op0=mult,
                        )
                        dst_t = ag_in[1, b // bpp]
                        roff = (b % bpp) * P
                        nc.sync.dma_start(out=dst_t[roff:roff + P, :], in_=gsb[:])

            # piece collectives fire as their 5-block spans complete
            for p in range(npieces):
                nc.gpsimd.collective_compute(
                    "AllGather", mybir.AluOpType.bypass,
                    ins=[ag_in[1, p][:]], outs=[ag_out[1, p][:]], replica_groups=rg,
                )

            # ------------- phases 2-4 share the resident W2 tile
            if phases >= 2:
                (maxP1, ncP1, ncE1), (maxP2, ncP2, ncE2) = metas
                with tc.tile_pool(name="ph234", bufs=1) as ph234:
                    w2_sb = ph234.tile([P, kh, go], f8)
                    w2_r = w2[:].rearrange("p (k n) -> p k n", k=kh)
                    for gstart, gn in fgroups:
                        nc.sync.dma_start(
                            out=w2_sb[:, :, gstart:gstart + gn],
                            in_=w2_r[:, :, gstart:gstart + gn],
                        )

                    # ----- phase 2: conv1 per block -> x2[b]; g2[b] = (x2[b] @ Wc2) * s2[b]
                    with tc.tile_pool(name="ph2", bufs=1) as ph2, \
                         tc.tile_pool(name="gat", bufs=2) as gat_p, \
                         tc.tile_pool(name="wsl", bufs=2) as wsl_p, \
                         tc.tile_pool(name="agg", bufs=2) as agg_p, \
                         tc.tile_pool(name="x2b", bufs=3) as x2b_p, \
                         tc.tile_pool(name="gout2", bufs=2) as gout2_p, \
                         tc.tile_pool(name="cps", bufs=2, space="PSUM") as cps_p, \
                         tc.tile_pool(name="tps", bufs=2, space="PSUM") as tps_p, \
                         tc.tile_pool(name="gps2", bufs=1, space="PSUM") as gps2_p:
                        wc2_sb = ph2.tile([P, kh, h], bf16)
                        nc.sync.dma_start(out=wc2_sb[:], in_=wc2[:].rearrange("p (k n) -> p k n", k=kh))
                        for b in range(nb):
                            gt = gat_p.tile([P, cpb, h], f8, tag="gt")
                            gather_block(gt, 1, idx1_sb, b, maxP1, ncP1, ncE1)
                            ws = wsl_p.tile([P, cpb, P], f8, tag="ws")
                            build_ws(ws, iota_w, dcol1_sb, b)
                            x2b = x2b_p.tile([P, kh, P], bf16, tag="x2b")
                            conv_block(gt, ws, ident, x2b, ncE1[b], cps_p, tps_p, agg_p)
                            ps2 = gps2_p.tile([P, h], f32, tag="g2ps")
                            for k in range(kh):
                                for hh in range(h // 512):
                                    nc.tensor.matmul(
                                        ps2[:, hh * 512:(hh + 1) * 512],
                                        lhsT=x2b[:, k, :],
                                        rhs=wc2_sb[:, k, hh * 512:(hh + 1) * 512],
                                        start=(k == 0),
                                        stop=(k == kh - 1),
                                    )
                            gsb = gout2_p.tile([P, h], f8, tag="gsb2")
                            nc.vector.tensor_scalar(
                                out=gsb[:], in0=ps2[:], scalar1=s_sb[:, nb + b:nb + b + 1],
                                scalar2=None, op0=mult,
                            )
                            dst_t = ag_in[2, b // bpp]
                            roff = (b % bpp) * P
                            nc.sync.dma_start(out=dst_t[roff:roff + P, :], in_=gsb[:])

                    for p in range(npieces):
                        nc.gpsimd.collective_compute(
                            "AllGather", mybir.AluOpType.bypass,
                            ins=[ag_in[2, p][:]], outs=[ag_out[2, p][:]], replica_groups=rg,
                        )

                    # ----- phase 3+4: conv2 per block -> x3[b]; out[b] = sigmoid(s3*(x3[b] @ W2))
                    if phases >= 3:
                        with tc.tile_pool(name="gat3", bufs=2) as gat3_p, \
                             tc.tile_pool(name="wsl3", bufs=2) as wsl3_p, \
                             tc.tile_pool(name="agg3", bufs=2) as agg3_p, \
                             tc.tile_pool(name="x3b", bufs=3) as x3b_p, \
                             tc.tile_pool(name="fout", bufs=3) as fout_p, \
                             tc.tile_pool(name="cps3", bufs=2, space="PSUM") as cps3_p, \
                             tc.tile_pool(name="tps3", bufs=2, space="PSUM") as tps3_p, \
                             tc.tile_pool(name="fps", bufs=2, space="PSUM") as fps_p:
                            for b in range(nb):
                                gt = gat3_p.tile([P, cpb, h], f8, tag="gt3")
                                gather_block(gt, 2, idx2_sb, b, maxP2, ncP2, ncE2)
                                ws = wsl3_p.tile([P, cpb, P], f8, tag="ws3")
                                build_ws(ws, iota_w, dcol2_sb, b)
                                x3b = x3b_p.tile([P, kh, P], f8, tag="x3b")
                                conv_block(gt, ws, ident, x3b, ncE2[b], cps3_p, tps3_p, agg3_p)
                                if phases >= 4:
                                    for gstart, gn in fgroups:
                                        o = fout_p.tile([P, 2048], bf16, tag="fo")
                                        for cs in range(0, gn, 512):
                                            cn = min(512, gn - cs)
                                            ps4 = fps_p.tile([P, 512], f32, tag="fps")
                                            for k2 in range(0, kh, 2):
                                                nc.tensor.matmul(
                                                    ps4[:, :cn],
                                                    lhsT=x3b[:, k2:k2 + 2, :],
                                                    rhs=w2_sb[:, k2:k2 + 2, gstart + cs:gstart + cs + cn],
                                                    start=(k2 == 0),
                                                    stop=(k2 == kh - 2),
                                                    perf_mode=DR,
                                                )
                                            nc.scalar.activation(
                                                out=o[:, cs:cs + cn], in_=ps4[:, :cn], func=Sigmoid,
                                                scale=s_sb[:, 2 * nb + b:2 * nb + b + 1],
                                            )
                                        nc.sync.dma_start(
                                            out=out_d[b * P:(b + 1) * P, gstart:gstart + gn],
                                            in_=o[:, :gn],
                                        )

    nc.compile()
    return nc


# ---------------------------------------------------------------- entry point

def _ensure_ntff_hook():
    """Register the axon NTFF profile hook if the image's antenv lacks it."""
    import contextlib
    import ctypes
    import sys
    import types

    try:
        from antenv.axon_hooks import get_axon_ntff_profile_hook  # noqa: F401
        return
    except ImportError:
        pass
    try:
        import antenv
    except ImportError:
        return
    mod = types.ModuleType("antenv.axon_hooks")
    holder = [None]
    mod.set_axon_ntff_profile_hook = lambda h: holder.__setitem__(0, h)
    mod.get_axon_ntff_profile_hook = lambda: holder[0]
    sys.modules["antenv.axon_hooks"] = mod
    antenv.axon_hooks = mod
    try:
        lib = ctypes.CDLL("/opt/axon/libaxon_pjrt.so")
    except OSError:
        return
    if not hasattr(lib, "axon_start_nrt_profile"):
        return
    lib.axon_start_nrt_profile.argtypes = [
        ctypes.POINTER(ctypes.c_int64),
        ctypes.c_size_t,
    ]
    lib.axon_start_nrt_profile.restype = ctypes.c_int64
    lib.axon_stop_nrt_profile.argtypes = [ctypes.c_char_p]
    lib.axon_stop_nrt_profile.restype = ctypes.c_int64

    @contextlib.contextmanager
    def _hook(output_dir, device_ids):
        import jax

        jax.devices()
        if device_ids:
            ids = (ctypes.c_int64 * len(device_ids))(*device_ids)
            rc = lib.axon_start_nrt_profile(ids, len(device_ids))
        else:
            rc = lib.axon_start_nrt_profile(None, 0)
        if rc != 0:
            raise RuntimeError(f"axon_start_nrt_profile rc={rc}")
        try:
            yield
        finally:
            n = lib.axon_stop_nrt_profile(str(output_dir).encode())
            print(f"profile: {n} file(s) written to {output_dir}", file=sys.stderr)

    holder[0] = _hook


def _run_hw(cfg, inputs, trace=False):
    if trace:
        _ensure_ntff_hook()
    cpb, metas, in_maps = prep_inputs(cfg, inputs)
    phases = int(os.environ.get("GNN_PHASES", "4"))
    nc = build_bass(cfg, cpb, metas, phases=phases)
    res = run_bass_kernel_spmd(nc, in_maps, core_ids=list(range(cfg.n_cores)), trace=trace)
    full = np.concatenate(
        [np.asarray(res.results[c]["out"]).astype(np.float32) for c in range(cfg.n_cores)],
        axis=0,
    )
    return full[: cfg.n_nodes], res


def kernel(**inputs) -> np.ndarray:
    trace = bool(int(os.environ.get("GNN_TRACE", "0")))
    out, res = _run_hw(FULL, inputs, trace=trace)
    if trace and res.exec_time_ns is not None:
        print(f"HW exec time: {res.exec_time_ns} ns")
    return out
